# revision 26
# baseline (speedup 1.0000x reference)
"""Distributed Trainium2 attention kernel (8 NeuronCores).

Strategy: tensor-parallel over heads for QKV projection + attention
(4 query heads + their 1 shared KV head per core), then an AllToAll
switches to row-sharding so each core computes the output projection for
its 512 rows with the full wo. Host reassembles rows. All matmuls run in
bf16 with fp32 PSUM accumulation.

Attention uses a "flipped" PV formulation: the exp'd score tile et
[keys, q] is the matmul STATIONARY and V the moving operand, with a ones
column appended to V (129-wide moving), so the attention output lands as
[q-partition, hd] with the softmax denominator as its 129th column. The
reciprocal is then a cheap [128,1]-per-partition op, normalization is a
per-partition tensor_scalar during the PSUM->SBUF copy, and a PE
transpose restores the [hd, q] layout the AllToAll needs. This keeps the
multi-microsecond single-lane reciprocal + gpsimd broadcast of the naive
formulation off the chain critical path entirely.

RoPE is applied in row-major layout via a host-side even/odd column
permutation of wq/wk (rotation becomes contiguous half-block arithmetic),
then q/k are transposed to [head_dim, rows] on the TensorEngine.

DMA discipline: x is loaded in [128, 512] tiles (1KB per partition row,
above the 512B descriptor-efficiency threshold), weights/x/rope tables
are spread over all four sequencer queues (sync/scalar/gpsimd/vector),
and the gpsimd queue (idle during attention) prefetches the output
projection's at/wo tiles behind the first AllToAll so phase D starts the
moment the second AllToAll lands.
"""

import numpy as np
import ml_dtypes

import concourse.bass as bass
import concourse.mybir as mybir
import concourse.tile as tile
from concourse import bacc
from concourse import bass_utils

B, S, D = 2, 2048, 4096
H, HKV, HD = 32, 8, 128
HD2 = HD // 2
NC = 8
HL = H // NC            # 4 local q heads per core
BS = B * S              # 4096 global rows
R = BS // NC            # 512 output rows per core
NRB = BS // 128         # 32 row blocks
NDT = D // 128          # 32 contraction tiles
SCALE = 1.0 / float(np.sqrt(HD))
BF = mybir.dt.bfloat16
F32 = mybir.dt.float32

PROFILE = False         # set by test.py for neuron-profile capture
TMPDIR = None           # set by test.py to keep the trace dir


def _emit(nc, tc, io):
    xT, wqkvT, woT, ccR, ssR, trim, iden, iden32, out = io

    engs3 = (nc.sync, nc.scalar, nc.gpsimd)

    with (
        tc.tile_pool(name="cbuf", bufs=1) as cbuf,
        tc.tile_pool(name="qbuf", bufs=1) as qbuf,
        tc.tile_pool(name="kvbuf", bufs=1) as kvbuf,
        tc.tile_pool(name="dram", bufs=1, space="DRAM") as dram,
        tc.tile_pool(name="ps", bufs=1, space="PSUM") as ps,
    ):
        # ---- long-lived SBUF state ----
        q_sb = qbuf.tile([128, HL * BS], BF, tag="q")     # col = h*4096 + row
        kT_sb = kvbuf.tile([128, BS], BF, tag="k")        # col = row
        v_sb = kvbuf.tile([128, BS], BF, tag="v")         # col = rb*128 + hd

        trim_sb = cbuf.tile([128, 128], F32, tag="tm")
        iden_sb = cbuf.tile([128, 128], BF, tag="idn")
        iden32_sb = cbuf.tile([128, 128], F32, tag="idn32")
        onec_sb = cbuf.tile([128, 1], BF, tag="onec")
        oner_sb = cbuf.tile([1, 128], BF, tag="oner")

        # one AllToAll per local head (fired as soon as that head's chains
        # drain) so phase D's inputs arrive progressively
        a2a_in = [dram.tile([BS // 4, R], BF, name=f"a2a_in{h}") for h in range(4)]
        a2a_out = [dram.tile([BS // 4, R], BF, name=f"a2a_out{h}") for h in range(4)]

        # ================= phase B: QKV projection + RoPE =================
        with (
            tc.tile_pool(name="wbuf", bufs=1) as wbuf,
            tc.tile_pool(name="xs", bufs=1) as xs,
            tc.tile_pool(name="cs", bufs=6) as cs,
            tc.tile_pool(name="ts", bufs=8) as ts,
        ):
            # resident QKV weights: col = dt*768 + [0:512 q | 512:640 k | 640:768 v]
            w_sb = wbuf.tile([128, NDT * 768], BF, tag="w")
            # x tiles: one [128, 4*512] quad covers 4 d-slices x 512 rows
            xg = [[None] * (NDT // 4) for _ in range(8)]

            def issue_xg(g, dq):
                t = xs.tile([128, 2048], BF, tag="x", bufs=16, name=f"x{g}_{dq}")
                src_ap = xT[dq * 512:(dq + 1) * 512, g * 512:(g + 1) * 512] \
                    .rearrange("(b p) c -> p b c", p=128)
                dst_ap = t[:].rearrange("p (b c) -> p b c", b=4)
                engs3[(g * 8 + dq + 1) % 3].dma_start(dst_ap, src_ap)
                xg[g][dq] = t

            nc.sync.dma_start(trim_sb[:], trim[:])
            nc.scalar.dma_start(iden_sb[:], iden[:])
            nc.gpsimd.dma_start(iden32_sb[:], iden32[:])
            nc.vector.memset(onec_sb[:], 1.0)
            nc.vector.memset(oner_sb[:], 1.0)
            # interleave weight + first x-group loads across the queues so
            # the PE can start on dt=0 almost immediately
            for dt in range(NDT):
                engs3[dt % 3].dma_start(
                    w_sb[:, dt * 768: dt * 768 + 768],
                    wqkvT[dt * 128: (dt + 1) * 128, :],
                )
                if dt % 4 == 3:
                    issue_xg(0, dt // 4)

            # rope tables: one [128, 1024] tile covers 4 row blocks
            csq = {}

            def issue_cs(q):
                cct = cs.tile([128, 1024], BF, tag="cc", bufs=3, name=f"cc{q}")
                engs3[q % 3].dma_start(cct[:], ccR[:, q * 1024: (q + 1) * 1024])
                sst = cs.tile([128, 1024], BF, tag="ss", bufs=3, name=f"ss{q}")
                engs3[(q + 1) % 3].dma_start(sst[:], ssR[:, q * 1024: (q + 1) * 1024])
                csq[q] = (cct, sst)

            issue_cs(0)

            # rope tails are emitted one rb late, behind rb+1's matmuls
            def b_rope_tail_q(rb, ps_q):
                if rb % 4 == 0 and rb // 4 + 1 < 8:
                    issue_cs(rb // 4 + 1)
                cq, sq = csq[rb // 4]
                cct = cq[:, (rb % 4) * 256: (rb % 4 + 1) * 256]
                sst = sq[:, (rb % 4) * 256: (rb % 4 + 1) * 256]

                qe = ps_q[:].rearrange("p (h d) -> p h d", d=128)[:, :, 0:HD2]
                qo = ps_q[:].rearrange("p (h d) -> p h d", d=128)[:, :, HD2:HD]
                t1 = ts.tile([128, 256], BF, tag="t")
                t2 = ts.tile([128, 256], BF, tag="t")
                t3 = ts.tile([128, 256], BF, tag="t")
                t4 = ts.tile([128, 256], BF, tag="t")
                nc.vector.tensor_mul(t1[:], qe, cct)
                nc.vector.tensor_mul(t2[:], qo, sst)
                nc.vector.tensor_mul(t3[:], qe, sst)
                nc.vector.tensor_mul(t4[:], qo, cct)
                qrot = ts.tile([128, 512], BF, tag="qr")
                qre = qrot[:].rearrange("p (h d) -> p h d", d=128)[:, :, 0:HD2]
                qro = qrot[:].rearrange("p (h d) -> p h d", d=128)[:, :, HD2:HD]
                nc.vector.tensor_sub(qre, t1[:], t2[:])
                nc.vector.tensor_add(qro, t3[:], t4[:])
                return (qrot, cct, sst)

            def b_transpose_tail_q(rb, qrot):
                ps_tq = ps.tile([128, 512], BF, tag="aux", bufs=2, padded_shape=[128, 1024])
                for h in range(HL):
                    nc.tensor.transpose(
                        ps_tq[:, h * 128: (h + 1) * 128],
                        qrot[:, h * 128: (h + 1) * 128],
                        iden_sb[:],
                    )
                q_dst = (
                    q_sb[:]
                    .rearrange("p (h r) -> p h r", h=HL)
                    [:, :, rb * 128: (rb + 1) * 128]
                )
                nc.vector.tensor_copy(
                    q_dst, ps_tq[:].rearrange("p (h r) -> p h r", h=HL)
                )

            def b_rope_tail_kv(rb, ps_kv, cct, sst):
                ke = ps_kv[:, 0:HD2]
                ko = ps_kv[:, HD2:HD]
                u1 = ts.tile([128, 64], BF, tag="u")
                u2 = ts.tile([128, 64], BF, tag="u")
                u3 = ts.tile([128, 64], BF, tag="u")
                u4 = ts.tile([128, 64], BF, tag="u")
                nc.vector.tensor_mul(u1[:], ke, cct[:, 0:HD2])
                nc.vector.tensor_mul(u2[:], ko, sst[:, 0:HD2])
                nc.vector.tensor_mul(u3[:], ke, sst[:, 0:HD2])
                nc.vector.tensor_mul(u4[:], ko, cct[:, 0:HD2])

                krot = ts.tile([128, 128], BF, tag="kr")
                nc.vector.tensor_sub(krot[:, 0:HD2], u1[:], u2[:])
                nc.vector.tensor_add(krot[:, HD2:HD], u3[:], u4[:])

                # v: plain copy to row-major storage
                nc.scalar.activation(
                    v_sb[:, rb * 128: (rb + 1) * 128], ps_kv[:, 128:256],
                    mybir.ActivationFunctionType.Copy,
                )
                return (krot,)

            def b_transpose_tail_kv(rb, krot):
                ps_tk = ps.tile([128, 128], BF, tag="aux", bufs=2, padded_shape=[128, 1024])
                nc.tensor.transpose(ps_tk[:], krot[:], iden_sb[:])
                nc.vector.tensor_copy(kT_sb[:, rb * 128: (rb + 1) * 128], ps_tk[:])

            pending = None
            rot = None
            for rb in range(NRB):
                g, ri = rb // 4, rb % 4
                ps_q = ps.tile([128, 512], F32, tag="pa", bufs=3)
                ps_kv = ps.tile([128, 256], F32, tag="s", bufs=3, padded_shape=[128, 512])
                for dt in range(NDT):
                    xt = xg[g][dt // 4][:, (dt % 4) * 512 + ri * 128:
                                        (dt % 4) * 512 + (ri + 1) * 128]
                    st, sp = dt == 0, dt == NDT - 1
                    nc.tensor.matmul(
                        ps_q[:], xt, w_sb[:, dt * 768: dt * 768 + 512],
                        start=st, stop=sp,
                    )
                    nc.tensor.matmul(
                        ps_kv[:], xt, w_sb[:, dt * 768 + 512: dt * 768 + 768],
                        start=st, stop=sp,
                    )
                    # prefetch next row-group's x quads, spread over this group
                    if ri == 2 and g + 1 < 8 and dt % 4 == 1:
                        issue_xg(g + 1, dt // 4)
                    if dt == 2 and pending is not None:
                        pq = b_rope_tail_q(pending[0], pending[1])
                        pkv = b_rope_tail_kv(pending[0], pending[2], pq[1], pq[2])
                        rot = (pending[0], pq[0]) + pkv
                        pending = None
                    if dt == 12 and rot is not None:
                        b_transpose_tail_q(rot[0], rot[1])
                        b_transpose_tail_kv(rot[0], rot[2])
                        rot = None
                pending = (rb, ps_q, ps_kv)
            pq = b_rope_tail_q(pending[0], pending[1])
            pkv = b_rope_tail_kv(pending[0], pending[2], pq[1], pq[2])
            b_transpose_tail_q(pending[0], pq[0])
            b_transpose_tail_kv(pending[0], pkv[0])

        # ============ phase C: causal attention (flipped PV) ============
        with (
            tc.tile_pool(name="es", bufs=6) as es,
            tc.tile_pool(name="rns", bufs=4) as rns,
            tc.tile_pool(name="abuf", bufs=1) as abuf,
            tc.tile_pool(name="ws", bufs=1) as ws,
            tc.tile_pool(name="osp", bufs=4) as osp,
        ):
            at_sb = abuf.tile([128, 32 * 512], BF, tag="at")  # col = ht*512+row
            # head-major so the earliest AllToAlls feed phase D's first
            # accumulation steps
            ht_order = [4 * i + l for l in range(4) for i in range(8)]
            wt0 = {}  # prefetched wo tiles for cg 0

            if True:

                def head_done(h):
                    """Fire head h's AllToAll + phase-D prefetches."""
                    nc.gpsimd.collective_compute(
                        "AllToAll",
                        mybir.AluOpType.bypass,
                        replica_groups=[list(range(NC))],
                        ins=[a2a_in[h].opt()],
                        outs=[a2a_out[h].opt()],
                    )
                    dst_ap = at_sb[:].rearrange(
                        "p (i c) -> p i c", c=512
                    )[:, h::4, :]
                    src_ap = a2a_out[h][:].rearrange("(i p) c -> p i c", p=128)
                    nc.gpsimd.dma_start(dst_ap, src_ap)
                    if h == 0:
                        for k in range(0, 32, 4):
                            wt = ws.tile([128, 2048], BF, tag="wo", bufs=16,
                                         name=f"wt0_{k}")
                            i0, lv = k % 8, k // 8
                            src_ap = woT[:].rearrange(
                                "(a l p) c -> p a l c", p=128, l=4
                            )[:, i0: i0 + 4, lv, 0:512]
                            nc.gpsimd.dma_start(
                                wt[:].rearrange("p (b c) -> p b c", b=4), src_ap
                            )
                            for n, ht in enumerate(ht_order[k: k + 4]):
                                wt0[ht] = wt[:, n * 512: (n + 1) * 512]

                def attn_chain(b, h, ci):
                    # j-major pipeline: scores -> exp -> PV (v-stationary,
                    # [hd, q] PSUM accumulation, one group per bank) with the
                    # softmax denominator accumulated on the DVE (acc += et).
                    qbase = h * BS + b * S
                    jmax = 4 * ci + 3
                    ps_attn = ps.tile([128, 512], F32, tag="pa", bufs=3,
                                       name=f"pa{b}{h}{ci}")
                    acc = rns.tile([128, 512], BF, tag="acc", bufs=3,
                                   name=f"acc{b}{h}{ci}")

                    def pv(j, et):
                        q0 = max(j * 128, 512 * ci)
                        w = 512 * ci + 512 - q0
                        off = q0 - 512 * ci
                        kcol = (b * 16 + j) * 128
                        nc.tensor.matmul(
                            ps_attn[:, off: off + w],
                            v_sb[:, kcol: kcol + 128],
                            et[:, 0:w],
                            start=(j == 0), stop=(j == jmax),
                        )
                        if j == 0:
                            nc.vector.tensor_copy(acc[:], et[:])
                        else:
                            nc.vector.tensor_add(
                                acc[:, off: off + w], acc[:, off: off + w],
                                et[:, 0:w],
                            )

                    prev = None
                    for j in range(jmax + 1):
                        q0 = max(j * 128, 512 * ci)
                        w = 512 * ci + 512 - q0
                        kcol = (b * 16 + j) * 128
                        ps_s = ps.tile([128, 512], F32, tag="s", bufs=3,
                                        name=f"s{b}{h}{ci}_{j}")
                        nc.tensor.matmul(
                            ps_s[:, 0:w],
                            kT_sb[:, kcol: kcol + 128],
                            q_sb[:, qbase + q0: qbase + q0 + w],
                            start=True, stop=True,
                        )
                        if j // 4 == ci:
                            nc.vector.tensor_add(
                                ps_s[:, 0:128], ps_s[:, 0:128], trim_sb[:]
                            )
                        et = es.tile([128, 512], BF, tag="e", bufs=6,
                                     name=f"e{b}{h}{ci}_{j}")
                        nc.scalar.activation(
                            et[:, 0:w], ps_s[:, 0:w],
                            mybir.ActivationFunctionType.Exp, scale=SCALE,
                        )
                        if prev is not None:
                            pv(*prev)
                        prev = (j, et)
                        yield
                    pv(*prev)

                    # tail: rowsums via 4 single-shot flipped matmuls on the
                    # DVE-accumulated acc, [128]-partition reciprocal, PE
                    # transpose of the reciprocals into a [1,512] row,
                    # broadcast, then one normalizing multiply.
                    rs4 = ps.tile([128, 4], F32, tag="aux", bufs=2,
                                   padded_shape=[128, 512],
                                   name=f"rs{b}{h}{ci}")
                    for qc in range(4):
                        nc.tensor.matmul(
                            rs4[:, qc: qc + 1],
                            acc[:, qc * 128: (qc + 1) * 128],
                            onec_sb[:],
                            start=True, stop=True,
                        )
                    rcT = rns.tile([128, 4], F32, tag="rc")
                    nc.vector.reciprocal(rcT[:], rs4[:])
                    psrc = ps.tile([1, 512], F32, tag="aux", bufs=2,
                                    padded_shape=[128, 512],
                                    name=f"pr{b}{h}{ci}")
                    for qc in range(4):
                        nc.tensor.transpose(
                            psrc[0:1, qc * 128: (qc + 1) * 128],
                            rcT[:, qc: qc + 1],
                            iden32_sb[:],
                        )
                    rc_row = rns.tile([1, 512], BF, tag="rcrow")
                    nc.scalar.activation(
                        rc_row[:], psrc[:], mybir.ActivationFunctionType.Copy
                    )
                    # broadcast on the PE: ones[1,128]^T (x) rc_row[1,512]
                    bc_ps = ps.tile([128, 512], F32, tag="aux", bufs=2,
                                     name=f"bc{b}{h}{ci}")
                    nc.tensor.matmul(
                        bc_ps[:], oner_sb[:], rc_row[:], start=True, stop=True
                    )
                    bc = rns.tile([128, 512], F32, tag="bc", bufs=2)
                    nc.vector.tensor_copy(bc[:], bc_ps[:])
                    an = rns.tile([128, 512], BF, tag="an")
                    nc.vector.tensor_mul(an[:], ps_attn[:], bc[:])
                    nc.sync.dma_start(
                        a2a_in[h][128 * (b * 4 + ci): 128 * (b * 4 + ci) + 128, :],
                        an[:],
                    )
                    yield

                # continuous 2-in-flight worklist; fire each head's AllToAll
                # the moment its last chain drains
                todo = [(b, h, ci)
                        for h in range(4) for b in range(B)
                        for ci in (0, 3, 1, 2)]
                todo.reverse()
                left = {h: 2 * 4 for h in range(4)}
                active = [[todo[-1][1], attn_chain(*todo.pop())],
                          [todo[-1][1], attn_chain(*todo.pop())],
                          [todo[-1][1], attn_chain(*todo.pop())]]
                while active:
                    for ent in list(active):
                        if next(ent[1], StopIteration) is StopIteration:
                            active.remove(ent)
                            left[ent[0]] -= 1
                            if left[ent[0]] == 0:
                                head_done(ent[0])
                            if todo:
                                active.append(
                                    [todo[-1][1], attn_chain(*todo.pop())]
                                )

            # ======== phase D: output projection for this core's rows ========
            if True:
                for cg in range(8):
                    po = [
                        ps.tile([128, 512], F32, tag=("pa" if i < 2 else "s"),
                                bufs=3, name=f"po{cg}_{i}")
                        for i in range(4)
                    ]
                    wtq = {}
                    for n_ht, ht in enumerate(ht_order):
                        if cg == 0:
                            wt = wt0[ht]
                        else:
                            if n_ht % 4 == 0:
                                wq4 = ws.tile([128, 2048], BF, tag="wo",
                                              bufs=16, name=f"wt{cg}_{n_ht}")
                                i0, lv = n_ht % 8, n_ht // 8
                                src_ap = woT[:].rearrange(
                                    "(a l p) c -> p a l c", p=128, l=4
                                )[:, i0: i0 + 4, lv,
                                  cg * 512: (cg + 1) * 512]
                                engs3[(n_ht // 4) % 3].dma_start(
                                    wq4[:].rearrange("p (b c) -> p b c", b=4),
                                    src_ap,
                                )
                                for n, ht2 in enumerate(ht_order[n_ht: n_ht + 4]):
                                    wtq[ht2] = wq4[:, n * 512: (n + 1) * 512]
                            wt = wtq[ht]
                        for rt in range(4):
                            nc.tensor.matmul(
                                po[rt][:],
                                at_sb[:, ht * 512 + rt * 128:
                                      ht * 512 + (rt + 1) * 128],
                                wt[:],
                                start=(n_ht == 0), stop=(n_ht == 31),
                            )
                    for rt in range(4):
                        ot = osp.tile([128, 512], F32, tag="o")
                        nc.vector.tensor_copy(ot[:], po[rt][:])
                        engs3[rt % 2].dma_start(
                            out[rt * 128: (rt + 1) * 128,
                                cg * 512: (cg + 1) * 512],
                            ot[:],
                        )


def _build():
    nc = bacc.Bacc("TRN2", target_bir_lowering=False, debug=False, num_devices=NC)
    xT = nc.dram_tensor("xT", [D, BS], BF, kind="ExternalInput")
    wqkvT = nc.dram_tensor("wqkvT", [D, 768], BF, kind="ExternalInput")
    woT = nc.dram_tensor("woT", [D, D], BF, kind="ExternalInput")
    ccR = nc.dram_tensor("ccR", [128, NRB * 256], BF, kind="ExternalInput")
    ssR = nc.dram_tensor("ssR", [128, NRB * 256], BF, kind="ExternalInput")
    trim = nc.dram_tensor("trim", [128, 128], F32, kind="ExternalInput")
    iden = nc.dram_tensor("iden", [128, 128], BF, kind="ExternalInput")
    iden32 = nc.dram_tensor("iden32", [128, 128], F32, kind="ExternalInput")
    out = nc.dram_tensor("out", [R, D], F32, kind="ExternalOutput")
    with tile.TileContext(nc) as tc:
        _emit(nc, tc, (xT, wqkvT, woT, ccR, ssR, trim, iden, iden32, out))
    nc.compile()
    return nc


_NC = None


def kernel(x, wq, wk, wv, wo, freqs_cos, freqs_sin, mask, start_pos):
    global _NC
    if _NC is None:
        _NC = _build()
    nc = _NC
    bf = ml_dtypes.bfloat16

    x = np.asarray(x, dtype=np.float32)
    xT = np.ascontiguousarray(x.reshape(BS, D).T).astype(bf)

    perm = np.concatenate([np.arange(0, HD, 2), np.arange(1, HD, 2)])
    wqTp = np.asarray(wq, np.float32).T.reshape(D, H, HD)[:, :, perm]
    wkTp = np.asarray(wk, np.float32).T.reshape(D, HKV, HD)[:, :, perm]
    wvT = np.asarray(wv, np.float32).T.reshape(D, HKV, HD)
    woT = np.ascontiguousarray(np.asarray(wo, np.float32).T).astype(bf)

    fc = np.asarray(freqs_cos, np.float32)
    fs = np.asarray(freqs_sin, np.float32)
    # row-major RoPE tables per row block, replicated x4 along free axis
    pos = (np.arange(BS) % S).reshape(NRB, 128)
    ccR = np.tile(fc[pos], (1, 1, 4)).transpose(1, 0, 2).reshape(128, NRB * 256)
    ssR = np.tile(fs[pos], (1, 1, 4)).transpose(1, 0, 2).reshape(128, NRB * 256)
    ccR = np.ascontiguousarray(ccR).astype(bf)
    ssR = np.ascontiguousarray(ssR).astype(bf)

    trim = np.where(
        np.arange(128)[:, None] > np.arange(128)[None, :], -1e30, 0.0
    ).astype(np.float32)
    iden = np.eye(128, dtype=bf)
    iden32 = np.eye(128, dtype=np.float32)

    in_maps = []
    for c in range(NC):
        wqkv = np.concatenate(
            [
                wqTp[:, 4 * c: 4 * c + 4].reshape(D, 512),
                wkTp[:, c],
                wvT[:, c],
            ],
            axis=1,
        ).astype(bf)
        in_maps.append(
            {
                "xT": xT,
                "wqkvT": np.ascontiguousarray(wqkv),
                "woT": woT,
                "ccR": ccR,
                "ssR": ssR,
                "trim": trim,
                "iden": iden,
                "iden32": iden32,
            }
        )

    res = bass_utils.run_bass_kernel_spmd(
        nc, in_maps, core_ids=list(range(NC)), trace=PROFILE, tmpdir=TMPDIR
    )
    if PROFILE:
        print(f"HW exec time: {res.exec_time_ns} ns")
        if res.instructions_and_trace is not None:
            print(f"trace: {res.instructions_and_trace[1]}")

    out_full = np.empty((BS, D), dtype=np.float32)
    for c in range(NC):
        out_full[R * c: R * (c + 1)] = res.results[c]["out"]
    return out_full.reshape(B, S, D)
